# revision 33
# baseline (speedup 1.0000x reference)
"""AttnBlock (GroupNorm -> qkv 1x1 -> softmax attention -> proj -> residual)
for x (2, 512, 64, 64) on 8 Trainium2 NeuronCores.

Sharding: core i handles batch i//4 and query-token block i%4 (1024 of 4096
spatial tokens). k/v are computed per-core over all 4096 tokens (no
collectives). Inputs are token-rolled per core so every core runs the same
SPMD graph with its own query block at token offset 0 (attention is
permutation-invariant over key/value tokens).

GroupNorm is folded into the projections: hn_c = a_c * x_c + d_c with
a_c = gamma_c * rsqrt(var_g + eps), d_c = beta_c - a_c * mu_g, so
q = (Wq diag(a)) x + (Wq d + bq), etc. The attention scale 1/sqrt(C) is
folded into Wq/Wk; k's bias cancels in softmax; v's bias folds into the
output bias, pre-added into the residual tile on the host. Only the fp8
copy of x is ever loaded (2MB): group sums run as indicator matmuls on the
otherwise-idle PE (which also keeps its clock ramped through the head),
and sums of squares split between ScalarE (2048-col Square+accum) and DVE
(mul+reduce), all chasing the DMA. Small inputs ride the Pool engine's DMA
queue so SP only carries x8.

The whole attention pipeline runs at the fp8 matmul rate with zero dtype
conversions: exp writes fp8 directly (shifted by -2 so p stays under
fp8e4's max finite 240; the shift cancels against the accumulated
denominator), the (i,j)->(j,i) transpose runs on the 2-byte DMA XBAR over
a uint16 *pair view* of the fp8 p, and the AV matmul consumes the pair
view via a byte-offset stride-2 DoubleRow lhsT (pair-plane stride 1 is
rejected by the ISA, stride-2 inner columns are fine). The byte pairing
lands token 128b+q (per 256-token block) on (partition q, byte b), so
k_sb is written column-reordered (strided copy out of PSUM) and each vw
tile n holds token block 2*(n%16) + n//16, making every lhsT/rhs pair
line up with no extra data movement. The first query block's QK/exp/
transpose are emitted before the v projection so its AV starts the moment
v lands.
"""

import numpy as np

C = 512          # channels
N = 4096         # spatial tokens (64*64)
NB = 1024        # query tokens per core
G = 32           # groups
CT = 4           # channel tiles of 128
EPS = 1e-6
SCALE = float(C) ** -0.5
QKSCALE = float(C) ** -0.25  # split between q and k so fp8 sees a good range
ESHIFT = -2.0    # exp(s + ESHIFT): keeps p < fp8e4 max finite (240)
NCORES = 8

_cache = {}


def _split_sync_waits(nc, maxw=1):
    """This walrus build encodes at most ~1 sync wait per instruction
    descriptor. Move excess sem waits onto same-engine nops inserted just
    before the instruction (in-order sequencers make this equivalent)."""
    from concourse import mybir

    n = 0
    for fn in nc.m.functions:
        for b in fn.blocks:
            out = []
            for ins in b.instructions:
                si = getattr(ins, "sync_info", None)
                if si is not None and si.on_wait and len(si.on_wait) > maxw:
                    waits = list(si.on_wait)
                    extra, keep = waits[:-maxw], waits[-maxw:]
                    for j in range(0, len(extra), maxw):
                        nop = mybir.InstNoOp(name=f"I-wsp{n}", ins=[], outs=[])
                        n += 1
                        nop.engine = ins.engine
                        nop.sync_info = mybir.SyncInfo(
                            on_wait=extra[j : j + maxw], on_update=[]
                        )
                        out.append(nop)
                    ins.sync_info = mybir.SyncInfo(
                        on_wait=keep, on_update=list(si.on_update)
                    )
                out.append(ins)
            b.instructions = out


def build(split_waits=True):
    import concourse.bass as bass
    import concourse.tile as tile
    from concourse import mybir

    f32 = mybir.dt.float32
    bf16 = mybir.dt.bfloat16
    fp8 = mybir.dt.float8e4
    u16 = mybir.dt.uint16
    AX = mybir.AxisListType
    ALU = mybir.AluOpType
    ACT = mybir.ActivationFunctionType
    DROW = mybir.MatmulPerfMode.DoubleRow

    nc = bass.Bass()
    X8 = nc.declare_dram_parameter("x_f8", [CT, 128, N], fp8, isOutput=False)
    WQ = nc.declare_dram_parameter("wq_t", [C, C], bf16, isOutput=False)
    WK = nc.declare_dram_parameter("wk_t", [C, C], bf16, isOutput=False)
    WOV = nc.declare_dram_parameter("wov_t", [C, C], bf16, isOutput=False)
    GAM = nc.declare_dram_parameter("gamma", [128, CT, 1], f32, isOutput=False)
    BET = nc.declare_dram_parameter("beta", [128, CT, 1], f32, isOutput=False)
    BQS = nc.declare_dram_parameter("bq_s", [128, CT, 1], f32, isOutput=False)
    GS8 = nc.declare_dram_parameter("gsum8", [128, CT, G], fp8, isOutput=False)
    GSF = nc.declare_dram_parameter("gsumf", [128, CT, G], f32, isOutput=False)
    GB = nc.declare_dram_parameter("gbcast", [G, CT, 128], f32, isOutput=False)
    XRT = nc.declare_dram_parameter("xres_t", [128, NB // 128, C], f32, isOutput=False)
    OUT = nc.declare_dram_parameter("out", [NB // 128, 128, C], bf16, isOutput=True)

    w_re = {
        "q": WQ.rearrange("(a p) o -> p a o", p=128),
        "k": WK.rearrange("(a p) o -> p a o", p=128),
        "ov": WOV.rearrange("(a p) o -> p a o", p=128),
    }

    with tile.TileContext(nc) as tc, (
        tc.tile_pool(name="singles", bufs=1)
    ) as singles, (
        tc.tile_pool(name="persist", bufs=1)
    ) as persist, (
        tc.tile_pool(name="ps_big", bufs=3, space="PSUM")
    ) as ps_big, (
        tc.tile_pool(name="xbp", bufs=1)
    ) as xbp, (
        tc.tile_pool(name="wfp", bufs=2)
    ) as wfp, (
        tc.tile_pool(name="statp", bufs=2)
    ) as statp, (
        tc.tile_pool(name="junkp", bufs=2)
    ) as junkp, (
        tc.tile_pool(name="junkv", bufs=2)
    ) as junkv:
        # persistent attention tensors
        xrt = persist.tile([128, NB // 128, C], f32)
        wq_s = persist.tile([128, CT, C], fp8)
        wk_s = persist.tile([128, CT, C], fp8)
        wov_s = persist.tile([128, CT, C], fp8)
        q_sb = persist.tile([128, CT, NB], fp8)
        k_sb = persist.tile([128, CT, N], fp8)
        vw_sb = persist.tile([128, N // 128, C], fp8)
        esh = singles.tile([128, 1], f32)

        with (
            tc.tile_pool(name="ps_st", bufs=1, space="PSUM") as ps_st,
            tc.tile_pool(name="ps_sg", bufs=1, space="PSUM") as ps_sg,
        ):
            # ---- fp8 x: the only copy of x ----
            x8 = xbp.tile([128, CT, N], fp8)
            for ct in range(CT):
                for h in range(2):
                    nc.sync.dma_start(
                        out=x8[:, ct, h * 2048 : (h + 1) * 2048],
                        in_=X8[ct, :, h * 2048 : (h + 1) * 2048],
                    )

            # ---- small inputs on the Pool DMA queue (SP carries only
            # x8); the stats matmul indicator goes first ----
            gs8_t = singles.tile([128, CT, G], fp8)
            nc.gpsimd.dma_start(out=gs8_t, in_=GS8[:, :, :])
            gsf_t = singles.tile([128, CT, G], f32)
            nc.gpsimd.dma_start(out=gsf_t, in_=GSF[:, :, :])
            gb_t = singles.tile([G, CT, 128], f32)
            nc.gpsimd.dma_start(out=gb_t, in_=GB[:, :, :])
            gam_t = singles.tile([128, CT, 1], f32)
            nc.gpsimd.dma_start(out=gam_t, in_=GAM[:, :, :])
            bet_t = singles.tile([128, CT, 1], f32)
            nc.gpsimd.dma_start(out=bet_t, in_=BET[:, :, :])
            bqs_t = singles.tile([128, CT, 1], f32)
            nc.gpsimd.dma_start(out=bqs_t, in_=BQS[:, :, :])
            nc.vector.memset(esh, ESHIFT)

            # group sums on the PE: psg[g, j] accumulates
            # (1/16) * sum_{c in g} x8[c, j (mod 512 partials)];
            # these 32 matmuls also keep the PE clock ramped.
            # sums of squares: ScalarE (2048-col Square+accum, late
            # chunks) / DVE (mul+reduce, early chunks) split.
            psg = ps_sg.tile([128, 512], f32, tag="psg")
            sums2 = statp.tile([128, CT, 4], f32, tag="sums2")
            nc.vector.memset(sums2, 0.0)  # 2048-col passes skip odd slots
            nmm = 0
            for ct in range(CT):
                for h in range(4):
                    chunk = x8[:, ct, h * 1024 : (h + 1) * 1024]
                    for c8 in range(2):
                        nc.tensor.matmul(
                            psg[:G, :],
                            gs8_t[:, ct, :],
                            chunk[:, c8 * 512 : (c8 + 1) * 512],
                            start=(nmm == 0),
                            stop=(nmm == 31),
                        )
                        nmm += 1
                    idx = ct * 4 + h
                    if idx < 6:
                        # DVE squares (its own junk pool: sharing one with
                        # ScalarE's serializes buffer reuse cross-engine)
                        junk8 = junkv.tile([128, 1024], fp8, tag="junk8")
                        nc.vector.tensor_mul(out=junk8, in0=chunk, in1=chunk)
                        nc.vector.reduce_sum(
                            out=sums2[:, ct, h : h + 1], in_=junk8, axis=AX.X
                        )
                    elif idx % 2 == 1:
                        junk = junkp.tile([128, 2048], bf16, tag="junk")
                        nc.scalar.activation(
                            out=junk,
                            in_=x8[:, ct, (h - 1) * 1024 : (h + 1) * 1024],
                            func=ACT.Square,
                            accum_out=sums2[:, ct, h : h + 1],
                        )

            # squares partials -> groups (gsf carries 1/16)
            ps2 = ps_st.tile([128, 4], f32, tag="ps_small")
            for ct in range(CT):
                nc.tensor.matmul(
                    ps2[:G, :],
                    gsf_t[:, ct, :],
                    sums2[:, ct, :],
                    start=(ct == 0),
                    stop=(ct == CT - 1),
                )
            # gst = [mu_g, E2_g] (means over the group's 16*4096 values)
            gst = statp.tile([G, 2], f32, tag="gst")
            nc.vector.reduce_sum(out=gst[:, 0:1], in_=psg[:G, :], axis=AX.X)
            nc.vector.reduce_sum(out=gst[:, 1:2], in_=ps2[:G, :], axis=AX.X)
            nc.scalar.mul(out=gst, in_=gst, mul=1.0 / N)
            gvar = statp.tile([G, 1], f32, tag="gvar")
            nc.vector.tensor_mul(out=gvar, in0=gst[:, 0:1], in1=gst[:, 0:1])
            nc.vector.tensor_sub(out=gvar, in0=gst[:, 1:2], in1=gvar)
            eps_t = statp.tile([G, 1], f32, tag="eps")
            nc.vector.memset(eps_t, EPS)
            gsq = statp.tile([G, 1], f32, tag="gsq")
            nc.scalar.activation(
                out=gsq, in_=gvar, func=ACT.Sqrt, bias=eps_t, scale=1.0
            )
            gstat2 = statp.tile([G, 2], f32, tag="gstat2")
            nc.vector.reciprocal(out=gstat2[:, 1:2], in_=gsq)
            nc.vector.tensor_copy(out=gstat2[:, 0:1], in_=gst[:, 0:1])

            # broadcast groups -> channels: mu_inv (128, CT, 2); the 4
            # matmuls land in disjoint columns of one PSUM tile (no WAR,
            # so they pipeline), then one strided copy extracts all four
            mu_inv = statp.tile([128, CT, 2], f32, tag="mu_inv")
            psb = ps_st.tile([128, CT, 4], f32, tag="ps_small")
            for ct in range(CT):
                nc.tensor.matmul(
                    psb[:, ct, 0:2], gb_t[:, ct, :], gstat2,
                    start=True, stop=True,
                )
            nc.vector.tensor_copy(out=mu_inv, in_=psb[:, :, 0:2])

            # a = gamma * inv ; d = beta - a * mu ; aq = a * SCALE
            a_t = statp.tile([128, CT, 1], f32, tag="a_t")
            nc.vector.tensor_mul(out=a_t, in0=gam_t, in1=mu_inv[:, :, 1:2])
            d_t = statp.tile([128, CT, 1], f32, tag="d_t")
            nc.vector.tensor_mul(out=d_t, in0=a_t, in1=mu_inv[:, :, 0:1])
            nc.vector.tensor_sub(out=d_t, in0=bet_t, in1=d_t)
            aq_t = statp.tile([128, CT, 1], f32, tag="aq_t")
            nc.scalar.mul(out=aq_t, in_=a_t, mul=QKSCALE)
            d_bf = statp.tile([128, CT, 1], bf16, tag="d_bf")
            nc.vector.tensor_copy(out=d_bf, in_=d_t)

            # stream q/k/v weights: fold + q bias projection
            bias_q = statp.tile([128, CT, 1], f32, tag="bias_q")
            for wname, wdst, scal, bvec, bdst, bscale in (
                ("q", wq_s, aq_t, d_bf, bias_q, QKSCALE),
                ("k", wk_s, aq_t, None, None, None),
                ("ov", wov_s, a_t, None, None, None),
            ):
                wf = wfp.tile([128, CT, C], bf16, tag="wf")
                nc.gpsimd.dma_start(out=wf, in_=w_re[wname])
                for ct in range(CT):
                    # fold split across DVE and ScalarE (per-partition
                    # scale rides the activation)
                    if ct < 2:
                        nc.vector.tensor_scalar_mul(
                            out=wdst[:, ct, :],
                            in0=wf[:, ct, :],
                            scalar1=scal[:, ct, :],
                        )
                    else:
                        nc.scalar.activation(
                            out=wdst[:, ct, :],
                            in_=wf[:, ct, :],
                            func=ACT.Copy,
                            scale=scal[:, ct, :],
                        )
                if bvec is not None:
                    for ot in range(CT):
                        pb = ps_st.tile([128, 4], f32, tag="ps_small")
                        for ct in range(CT):
                            nc.tensor.matmul(
                                pb[:, 0:1],
                                wf[:, ct, ot * 128 : (ot + 1) * 128],
                                bvec[:, ct, :],
                                start=(ct == 0),
                                stop=(ct == CT - 1),
                            )
                        nc.scalar.activation(
                            out=bdst[:, ot, :],
                            in_=pb[:, 0:1],
                            func=ACT.Identity,
                            bias=bqs_t[:, ot, :],
                            scale=bscale,
                        )

        # ---- projections + attention (small-PSUM pools closed above
        # free the banks ps_av needs) ----
        with (
            tc.tile_pool(name="loopp", bufs=4) as loopp,
            tc.tile_pool(name="sblk", bufs=4) as sblk,
            tc.tile_pool(name="ps_av", bufs=2, space="PSUM") as ps_av,
        ):
            # q: per ot one [128, 1024] PSUM, bias applied on read-out
            for ot in range(CT):
                ps = ps_big.tile([128, 1024], f32, tag="psbig")
                for jc in range(2):
                    for c2 in range(2):
                        nc.tensor.matmul(
                            ps[:, jc * 512 : (jc + 1) * 512],
                            wq_s[:, 2 * c2 : 2 * c2 + 2, ot * 128 : (ot + 1) * 128],
                            x8[:, 2 * c2 : 2 * c2 + 2, jc * 512 : (jc + 1) * 512],
                            start=(c2 == 0),
                            stop=(c2 == 1),
                            perf_mode=DROW,
                        )
                nc.scalar.activation(
                    out=q_sb[:, ot, :],
                    in_=ps,
                    func=ACT.Identity,
                    bias=bias_q[:, ot, :],
                    scale=1.0,
                )

            # k: [128, 1024] PSUM chunks; copy out column-reordered so
            # k_sb column 256B + 2t + b holds token 256B + 128b + t
            # (matches the uint16 pair transpose in the AV stage)
            for ot in range(CT):
                for j2 in range(N // 1024):
                    ps = ps_big.tile([128, 1024], f32, tag="psbig")
                    for jc in range(2):
                        for c2 in range(2):
                            nc.tensor.matmul(
                                ps[:, jc * 512 : (jc + 1) * 512],
                                wk_s[:, 2 * c2 : 2 * c2 + 2, ot * 128 : (ot + 1) * 128],
                                x8[
                                    :,
                                    2 * c2 : 2 * c2 + 2,
                                    j2 * 1024 + jc * 512 : j2 * 1024 + (jc + 1) * 512,
                                ],
                                start=(c2 == 0),
                                stop=(c2 == 1),
                                perf_mode=DROW,
                            )
                    dst = k_sb[:, ot, j2 * 1024 : (j2 + 1) * 1024].rearrange(
                        "p (blk t two) -> p blk two t", blk=4, t=128, two=2
                    )
                    if j2 % 2 == 0:
                        nc.scalar.activation(out=dst, in_=ps, func=ACT.Copy)
                    else:
                        nc.vector.tensor_copy(out=dst, in_=ps)

            def qk_part(ib, fine=False):
                """Scores, exp->fp8, denominator, pair transposes."""
                i0 = ib * 128
                p8 = sblk.tile([128, N], fp8, tag="p8")
                denp = loopp.tile([128, 4], f32, tag="denp")
                pT16 = sblk.tile([128, N // 256, 128], u16, tag="pT16")
                p16 = p8[:, :].bitcast(u16)
                for jc in range(N // 1024):
                    ps = ps_big.tile([128, 1024], f32, tag="psbig")
                    for half in range(2):
                        for c2 in range(2):
                            nc.tensor.matmul(
                                ps[:, half * 512 : (half + 1) * 512],
                                q_sb[:, 2 * c2 : 2 * c2 + 2, i0 : i0 + 128],
                                k_sb[
                                    :,
                                    2 * c2 : 2 * c2 + 2,
                                    jc * 1024 + half * 512 : jc * 1024 + (half + 1) * 512,
                                ],
                                start=(c2 == 0),
                                stop=(c2 == 1),
                                perf_mode=DROW,
                            )
                    nc.scalar.activation(
                        out=p8[:, jc * 1024 : (jc + 1) * 1024],
                        in_=ps,
                        func=ACT.Exp,
                        bias=esh,
                        scale=1.0,
                        accum_out=denp[:, jc : jc + 1],
                    )
                    # uint16 pair-view transpose: byte b of u16 (q, t16, i)
                    # holds p[i, col 2*(128*t16+q)+b]; first chunk alone so
                    # the (sequential) AV accumulation starts early. For the
                    # tail blocks, per-chunk transposes shorten the drain.
                    if fine or jc == 0:
                        nc.sync.dma_start_transpose(
                            pT16[:, 4 * jc : 4 * jc + 4, :],
                            p16[:, jc * 512 : (jc + 1) * 512],
                        )
                    elif jc == 3:
                        nc.sync.dma_start_transpose(
                            pT16[:, 4:16, :], p16[:, 512:2048]
                        )
                den = loopp.tile([128, 1], f32, tag="den")
                nc.vector.reduce_sum(out=den, in_=denp, axis=AX.X)
                rden = loopp.tile([128, 1], f32, tag="rden")
                nc.vector.reciprocal(out=rden, in_=den)
                return pT16, rden

            def av_part(ib, pT16, rden):
                """fp8 DoubleRow AV from the pair view, residual, out."""
                pav = ps_av.tile([128, C], f32, tag="pav")
                pT8v = pT16[:, :, :].bitcast(fp8)  # [128, 16, 256]
                nmm = 0
                for u in range(8):
                    for b in range(2):
                        lhs = pT8v[:, 2 * u : 2 * u + 2, :].rearrange(
                            "p t (i two) -> p t two i", two=2
                        )[:, :, b, :]
                        nc.tensor.matmul(
                            pav,
                            lhs,
                            vw_sb[:, 16 * b + 2 * u : 16 * b + 2 * u + 2, :],
                            start=(nmm == 0),
                            stop=(nmm == 15),
                            perf_mode=DROW,
                        )
                        nmm += 1
                outf = loopp.tile([128, C], bf16, tag="outf")
                nc.vector.scalar_tensor_tensor(
                    out=outf,
                    in0=pav,
                    scalar=rden,
                    in1=xrt[:, ib, :],
                    op0=ALU.mult,
                    op1=ALU.add,
                )
                nc.sync.dma_start(out=OUT[ib], in_=outf)

            # v: tile n <- token block 2*(n%16) + n//16; adjacent n pairs
            # share one [128, 1024] PSUM so copies run at 1024 cols
            for n2 in range(N // 256):
                ps = ps_big.tile([128, 1024], f32, tag="psbig")
                for half in range(2):
                    n = 2 * n2 + half
                    g = 2 * (n % 16) + n // 16
                    for c2 in range(2):
                        nc.tensor.matmul(
                            ps[:, half * 512 : (half + 1) * 512],
                            x8[:, 2 * c2 : 2 * c2 + 2, g * 128 : (g + 1) * 128],
                            wov_s[:, 2 * c2 : 2 * c2 + 2, :],
                            start=(c2 == 0),
                            stop=(c2 == 1),
                            perf_mode=DROW,
                        )
                dst = vw_sb[:, 2 * n2 : 2 * n2 + 2, :]
                if n2 % 2 == 0:
                    nc.scalar.activation(out=dst, in_=ps, func=ACT.Copy)
                else:
                    nc.vector.tensor_copy(out=dst, in_=ps)

            # token-major residual (output bias pre-added on host)
            nc.gpsimd.dma_start(out=xrt, in_=XRT[:, :, :])

            for ib in range(NB // 128):
                pT16_i, rden_i = qk_part(ib, fine=(ib >= 6))
                av_part(ib, pT16_i, rden_i)

    if split_waits:
        _split_sync_waits(nc)
    return nc


def _prep_in_maps(x, gn_gamma, gn_beta, wq, bq, wk, bk, wv, bv, wo, bo):
    import ml_dtypes

    f = np.float32
    bf = ml_dtypes.bfloat16
    xr = np.asarray(x, f).reshape(2, C, N)
    wq_t = np.ascontiguousarray(np.asarray(wq, f).T.astype(bf))
    wk_t = np.ascontiguousarray(np.asarray(wk, f).T.astype(bf))
    wov_t = np.ascontiguousarray((np.asarray(wo, f) @ np.asarray(wv, f)).T.astype(bf))
    bias_o0 = np.asarray(bo, f) + np.asarray(wo, f) @ np.asarray(bv, f)

    f8 = ml_dtypes.float8_e4m3  # matches mybir.dt.float8e4's layout

    def vec(v, dt=f):
        return np.ascontiguousarray(
            np.asarray(v, f).reshape(CT, 128).transpose(1, 0)[:, :, None].astype(dt)
        )

    gam = vec(gn_gamma)
    bet = vec(gn_beta)
    bq_s = vec(np.asarray(bq, f) * QKSCALE)

    cidx = np.arange(C)
    grp = cidx // 16  # (512,)
    gsum = np.zeros((128, CT, G), f)
    gbcast = np.zeros((G, CT, 128), f)
    for ct in range(CT):
        for cl in range(128):
            g = grp[ct * 128 + cl]
            gsum[cl, ct, g] = 1.0 / 16.0  # averages the group's channels
            gbcast[g, ct, cl] = 1.0

    in_maps = []
    for core in range(NCORES):
        b, r = divmod(core, 4)
        xroll = np.ascontiguousarray(np.roll(xr[b], -r * NB, axis=1).reshape(CT, 128, N))
        xres_t = np.ascontiguousarray(
            (xroll.reshape(C, N)[:, :NB].T + bias_o0[None, :])
            .reshape(NB // 128, 128, C)
            .transpose(1, 0, 2)
        )
        in_maps.append(
            {
                "x_f8": xroll.astype(f8),
                "xres_t": xres_t,
                "wq_t": wq_t,
                "wk_t": wk_t,
                "wov_t": wov_t,
                "gamma": gam,
                "beta": bet,
                "bq_s": bq_s,
                "gsum8": gsum.astype(f8),
                "gsumf": gsum,
                "gbcast": gbcast,
            }
        )
    return in_maps


def _assemble(results):
    out = np.empty((2, C, N), np.float32)
    for core in range(NCORES):
        b, r = divmod(core, 4)
        out[b][:, r * NB : (r + 1) * NB] = (
            np.asarray(results[core]["out"]).astype(np.float32).reshape(NB, C).T
        )
    return out.reshape(2, C, 64, 64)


def _run(in_maps, trace=False, trace_kwargs=None):
    from concourse.bass_utils import run_bass_kernel_spmd

    if "nc" not in _cache:
        _cache["nc"] = build()
    kw = {}
    if trace:
        kw = {"trace": True, "trace_kwargs": trace_kwargs or {}}
    return run_bass_kernel_spmd(
        _cache["nc"], in_maps, core_ids=list(range(NCORES)), **kw
    )


def kernel(x, gn_gamma, gn_beta, wq, bq, wk, bk, wv, bv, wo, bo):
    in_maps = _prep_in_maps(x, gn_gamma, gn_beta, wq, bq, wk, bk, wv, bv, wo, bo)
    res = _run(in_maps, trace=False)
    return _assemble(res.results)


# revision 34
# speedup vs baseline: 1.0217x; 1.0217x over previous
"""AttnBlock (GroupNorm -> qkv 1x1 -> softmax attention -> proj -> residual)
for x (2, 512, 64, 64) on 8 Trainium2 NeuronCores.

Sharding: core i handles batch i//4 and query-token block i%4 (1024 of 4096
spatial tokens). k/v are computed per-core over all 4096 tokens (no
collectives). Inputs are token-rolled per core so every core runs the same
SPMD graph with its own query block at token offset 0 (attention is
permutation-invariant over key/value tokens).

GroupNorm is folded into the projections: hn_c = a_c * x_c + d_c with
a_c = gamma_c * rsqrt(var_g + eps), d_c = beta_c - a_c * mu_g, so
q = (Wq diag(a)) x + (Wq d + bq), etc. The attention scale 1/sqrt(C) is
folded into Wq/Wk; k's bias cancels in softmax; v's bias folds into the
output bias, pre-added into the residual tile on the host. Only the fp8
copy of x is ever loaded (2MB): group sums run as indicator matmuls on the
otherwise-idle PE (which also keeps its clock ramped through the head),
and sums of squares split between ScalarE (2048-col Square+accum) and DVE
(mul+reduce), all chasing the DMA. Small inputs ride the Pool engine's DMA
queue so SP only carries x8.

The whole attention pipeline runs at the fp8 matmul rate with zero dtype
conversions: exp writes fp8 directly (shifted by -2 so p stays under
fp8e4's max finite 240; the shift cancels against the accumulated
denominator), the (i,j)->(j,i) transpose runs on the 2-byte DMA XBAR over
a uint16 *pair view* of the fp8 p, and the AV matmul consumes the pair
view via a byte-offset stride-2 DoubleRow lhsT (pair-plane stride 1 is
rejected by the ISA, stride-2 inner columns are fine). The byte pairing
lands token 128b+q (per 256-token block) on (partition q, byte b), so
k_sb is written column-reordered (strided copy out of PSUM) and each vw
tile n holds token block 2*(n%16) + n//16, making every lhsT/rhs pair
line up with no extra data movement. The first query block's QK/exp/
transpose are emitted before the v projection so its AV starts the moment
v lands.
"""

import numpy as np

C = 512          # channels
N = 4096         # spatial tokens (64*64)
NB = 1024        # query tokens per core
G = 32           # groups
CT = 4           # channel tiles of 128
EPS = 1e-6
SCALE = float(C) ** -0.5
QKSCALE = float(C) ** -0.25  # split between q and k so fp8 sees a good range
ESHIFT = -2.0    # exp(s + ESHIFT): keeps p < fp8e4 max finite (240)
NCORES = 8

_cache = {}


def _split_sync_waits(nc, maxw=1):
    """This walrus build encodes at most ~1 sync wait per instruction
    descriptor. Move excess sem waits onto same-engine nops inserted just
    before the instruction (in-order sequencers make this equivalent)."""
    from concourse import mybir

    n = 0
    for fn in nc.m.functions:
        for b in fn.blocks:
            out = []
            for ins in b.instructions:
                si = getattr(ins, "sync_info", None)
                if si is not None and si.on_wait and len(si.on_wait) > maxw:
                    waits = list(si.on_wait)
                    extra, keep = waits[:-maxw], waits[-maxw:]
                    for j in range(0, len(extra), maxw):
                        nop = mybir.InstNoOp(name=f"I-wsp{n}", ins=[], outs=[])
                        n += 1
                        nop.engine = ins.engine
                        nop.sync_info = mybir.SyncInfo(
                            on_wait=extra[j : j + maxw], on_update=[]
                        )
                        out.append(nop)
                    ins.sync_info = mybir.SyncInfo(
                        on_wait=keep, on_update=list(si.on_update)
                    )
                out.append(ins)
            b.instructions = out


def build(split_waits=True):
    import concourse.bass as bass
    import concourse.tile as tile
    from concourse import mybir

    f32 = mybir.dt.float32
    bf16 = mybir.dt.bfloat16
    fp8 = mybir.dt.float8e4
    u16 = mybir.dt.uint16
    AX = mybir.AxisListType
    ALU = mybir.AluOpType
    ACT = mybir.ActivationFunctionType
    DROW = mybir.MatmulPerfMode.DoubleRow

    nc = bass.Bass()
    X8 = nc.declare_dram_parameter("x_f8", [CT, 128, N], fp8, isOutput=False)
    WQ = nc.declare_dram_parameter("wq_t", [C, C], bf16, isOutput=False)
    WK = nc.declare_dram_parameter("wk_t", [C, C], bf16, isOutput=False)
    WOV = nc.declare_dram_parameter("wov_t", [C, C], bf16, isOutput=False)
    GAM = nc.declare_dram_parameter("gamma", [128, CT, 1], f32, isOutput=False)
    BET = nc.declare_dram_parameter("beta", [128, CT, 1], f32, isOutput=False)
    BQS = nc.declare_dram_parameter("bq_s", [128, CT, 1], f32, isOutput=False)
    GS8 = nc.declare_dram_parameter("gsum8", [128, CT, G], fp8, isOutput=False)
    GSF = nc.declare_dram_parameter("gsumf", [128, CT, G], f32, isOutput=False)
    GB = nc.declare_dram_parameter("gbcast", [G, CT, 128], f32, isOutput=False)
    XRT = nc.declare_dram_parameter("xres_t", [128, NB // 128, C], f32, isOutput=False)
    OUT = nc.declare_dram_parameter("out", [NB // 128, 128, C], bf16, isOutput=True)

    w_re = {
        "q": WQ.rearrange("(a p) o -> p a o", p=128),
        "k": WK.rearrange("(a p) o -> p a o", p=128),
        "ov": WOV.rearrange("(a p) o -> p a o", p=128),
    }

    with tile.TileContext(nc) as tc, (
        tc.tile_pool(name="singles", bufs=1)
    ) as singles, (
        tc.tile_pool(name="persist", bufs=1)
    ) as persist, (
        tc.tile_pool(name="ps_big", bufs=3, space="PSUM")
    ) as ps_big, (
        tc.tile_pool(name="xbp", bufs=1)
    ) as xbp, (
        tc.tile_pool(name="wfp", bufs=2)
    ) as wfp, (
        tc.tile_pool(name="statp", bufs=2)
    ) as statp, (
        tc.tile_pool(name="junkp", bufs=2)
    ) as junkp, (
        tc.tile_pool(name="junkv", bufs=2)
    ) as junkv:
        # persistent attention tensors
        xrt = persist.tile([128, NB // 128, C], f32)
        wq_s = persist.tile([128, CT, C], fp8)
        wk_s = persist.tile([128, CT, C], fp8)
        wov_s = persist.tile([128, CT, C], fp8)
        q_sb = persist.tile([128, CT, NB], fp8)
        k_sb = persist.tile([128, CT, N], fp8)
        vw_sb = persist.tile([128, N // 128, C], fp8)
        esh = singles.tile([128, 1], f32)

        with (
            tc.tile_pool(name="ps_st", bufs=1, space="PSUM") as ps_st,
            tc.tile_pool(name="ps_sg", bufs=1, space="PSUM") as ps_sg,
        ):
            # ---- fp8 x: the only copy of x ----
            x8 = xbp.tile([128, CT, N], fp8)
            for ct in range(CT):
                for h in range(2):
                    nc.sync.dma_start(
                        out=x8[:, ct, h * 2048 : (h + 1) * 2048],
                        in_=X8[ct, :, h * 2048 : (h + 1) * 2048],
                    )

            # ---- small inputs on the Pool DMA queue (SP carries only
            # x8); the stats matmul indicator goes first ----
            gs8_t = singles.tile([128, CT, G], fp8)
            nc.gpsimd.dma_start(out=gs8_t, in_=GS8[:, :, :])
            gsf_t = singles.tile([128, CT, G], f32)
            nc.gpsimd.dma_start(out=gsf_t, in_=GSF[:, :, :])
            gb_t = singles.tile([G, CT, 128], f32)
            nc.gpsimd.dma_start(out=gb_t, in_=GB[:, :, :])
            gam_t = singles.tile([128, CT, 1], f32)
            nc.gpsimd.dma_start(out=gam_t, in_=GAM[:, :, :])
            bet_t = singles.tile([128, CT, 1], f32)
            nc.gpsimd.dma_start(out=bet_t, in_=BET[:, :, :])
            bqs_t = singles.tile([128, CT, 1], f32)
            nc.gpsimd.dma_start(out=bqs_t, in_=BQS[:, :, :])
            nc.vector.memset(esh, ESHIFT)

            # group sums on the PE: psg[g, j] accumulates
            # (1/16) * sum_{c in g} x8[c, j (mod 512 partials)];
            # these 32 matmuls also keep the PE clock ramped.
            # sums of squares: ScalarE (2048-col Square+accum, late
            # chunks) / DVE (mul+reduce, early chunks) split.
            psg = ps_sg.tile([128, 512], f32, tag="psg")
            sums2 = statp.tile([128, CT, 4], f32, tag="sums2")
            nc.vector.memset(sums2, 0.0)  # 2048-col passes skip odd slots
            nmm = 0
            for ct in range(CT):
                for h in range(4):
                    chunk = x8[:, ct, h * 1024 : (h + 1) * 1024]
                    for c8 in range(2):
                        nc.tensor.matmul(
                            psg[:G, :],
                            gs8_t[:, ct, :],
                            chunk[:, c8 * 512 : (c8 + 1) * 512],
                            start=(nmm == 0),
                            stop=(nmm == 31),
                        )
                        nmm += 1
                    idx = ct * 4 + h
                    if idx < 6:
                        # DVE squares (its own junk pool: sharing one with
                        # ScalarE's serializes buffer reuse cross-engine)
                        junk8 = junkv.tile([128, 1024], fp8, tag="junk8")
                        nc.vector.tensor_mul(out=junk8, in0=chunk, in1=chunk)
                        nc.vector.reduce_sum(
                            out=sums2[:, ct, h : h + 1], in_=junk8, axis=AX.X
                        )
                    elif idx % 2 == 1:
                        junk = junkp.tile([128, 2048], bf16, tag="junk")
                        nc.scalar.activation(
                            out=junk,
                            in_=x8[:, ct, (h - 1) * 1024 : (h + 1) * 1024],
                            func=ACT.Square,
                            accum_out=sums2[:, ct, h : h + 1],
                        )

            # squares partials -> groups (gsf carries 1/16)
            ps2 = ps_st.tile([128, 4], f32, tag="ps_small")
            for ct in range(CT):
                nc.tensor.matmul(
                    ps2[:G, :],
                    gsf_t[:, ct, :],
                    sums2[:, ct, :],
                    start=(ct == 0),
                    stop=(ct == CT - 1),
                )
            # gst = [mu_g, E2_g] (means over the group's 16*4096 values)
            gst = statp.tile([G, 2], f32, tag="gst")
            nc.vector.reduce_sum(out=gst[:, 0:1], in_=psg[:G, :], axis=AX.X)
            nc.vector.reduce_sum(out=gst[:, 1:2], in_=ps2[:G, :], axis=AX.X)
            nc.scalar.mul(out=gst, in_=gst, mul=1.0 / N)
            gvar = statp.tile([G, 1], f32, tag="gvar")
            nc.vector.tensor_mul(out=gvar, in0=gst[:, 0:1], in1=gst[:, 0:1])
            nc.vector.tensor_sub(out=gvar, in0=gst[:, 1:2], in1=gvar)
            eps_t = statp.tile([G, 1], f32, tag="eps")
            nc.vector.memset(eps_t, EPS)
            gsq = statp.tile([G, 1], f32, tag="gsq")
            nc.scalar.activation(
                out=gsq, in_=gvar, func=ACT.Sqrt, bias=eps_t, scale=1.0
            )
            gstat2 = statp.tile([G, 2], f32, tag="gstat2")
            nc.vector.reciprocal(out=gstat2[:, 1:2], in_=gsq)
            nc.vector.tensor_copy(out=gstat2[:, 0:1], in_=gst[:, 0:1])

            # broadcast groups -> channels: mu_inv (128, CT, 2); the 4
            # matmuls land in disjoint columns of one PSUM tile (no WAR,
            # so they pipeline), then one strided copy extracts all four
            mu_inv = statp.tile([128, CT, 2], f32, tag="mu_inv")
            psb = ps_st.tile([128, CT, 4], f32, tag="ps_small")
            for ct in range(CT):
                nc.tensor.matmul(
                    psb[:, ct, 0:2], gb_t[:, ct, :], gstat2,
                    start=True, stop=True,
                )
            nc.vector.tensor_copy(out=mu_inv, in_=psb[:, :, 0:2])

            # a = gamma * inv ; d = beta - a * mu ; aq = a * SCALE
            a_t = statp.tile([128, CT, 1], f32, tag="a_t")
            nc.vector.tensor_mul(out=a_t, in0=gam_t, in1=mu_inv[:, :, 1:2])
            d_t = statp.tile([128, CT, 1], f32, tag="d_t")
            nc.vector.tensor_mul(out=d_t, in0=a_t, in1=mu_inv[:, :, 0:1])
            nc.vector.tensor_sub(out=d_t, in0=bet_t, in1=d_t)
            aq_t = statp.tile([128, CT, 1], f32, tag="aq_t")
            nc.scalar.mul(out=aq_t, in_=a_t, mul=QKSCALE)
            d_bf = statp.tile([128, CT, 1], bf16, tag="d_bf")
            nc.vector.tensor_copy(out=d_bf, in_=d_t)

            # stream q/k/v weights: fold + q bias projection
            bias_q = statp.tile([128, CT, 1], f32, tag="bias_q")
            for wname, wdst, scal, bvec, bdst, bscale in (
                ("q", wq_s, aq_t, d_bf, bias_q, QKSCALE),
                ("k", wk_s, aq_t, None, None, None),
                ("ov", wov_s, a_t, None, None, None),
            ):
                wf = wfp.tile([128, CT, C], bf16, tag="wf")
                nc.gpsimd.dma_start(out=wf, in_=w_re[wname])
                for ct in range(CT):
                    # fold split across DVE and ScalarE (per-partition
                    # scale rides the activation)
                    if ct < 2:
                        nc.vector.tensor_scalar_mul(
                            out=wdst[:, ct, :],
                            in0=wf[:, ct, :],
                            scalar1=scal[:, ct, :],
                        )
                    else:
                        nc.scalar.activation(
                            out=wdst[:, ct, :],
                            in_=wf[:, ct, :],
                            func=ACT.Copy,
                            scale=scal[:, ct, :],
                        )
                if bvec is not None:
                    for ot in range(CT):
                        pb = ps_st.tile([128, 4], f32, tag="ps_small")
                        for ct in range(CT):
                            nc.tensor.matmul(
                                pb[:, 0:1],
                                wf[:, ct, ot * 128 : (ot + 1) * 128],
                                bvec[:, ct, :],
                                start=(ct == 0),
                                stop=(ct == CT - 1),
                            )
                        nc.scalar.activation(
                            out=bdst[:, ot, :],
                            in_=pb[:, 0:1],
                            func=ACT.Identity,
                            bias=bqs_t[:, ot, :],
                            scale=bscale,
                        )

        # ---- projections + attention (small-PSUM pools closed above
        # free the banks ps_av needs) ----
        with (
            tc.tile_pool(name="loopp", bufs=3) as loopp,
            tc.tile_pool(name="sblk", bufs=3) as sblk,
            tc.tile_pool(name="ps_av", bufs=2, space="PSUM") as ps_av,
        ):
            # q: per ot one [128, 1024] PSUM, bias applied on read-out
            for ot in range(CT):
                ps = ps_big.tile([128, 1024], f32, tag="psbig")
                for jc in range(2):
                    for c2 in range(2):
                        nc.tensor.matmul(
                            ps[:, jc * 512 : (jc + 1) * 512],
                            wq_s[:, 2 * c2 : 2 * c2 + 2, ot * 128 : (ot + 1) * 128],
                            x8[:, 2 * c2 : 2 * c2 + 2, jc * 512 : (jc + 1) * 512],
                            start=(c2 == 0),
                            stop=(c2 == 1),
                            perf_mode=DROW,
                        )
                nc.scalar.activation(
                    out=q_sb[:, ot, :],
                    in_=ps,
                    func=ACT.Identity,
                    bias=bias_q[:, ot, :],
                    scale=1.0,
                )

            # k: [128, 1024] PSUM chunks; copy out column-reordered so
            # k_sb column 256B + 2t + b holds token 256B + 128b + t
            # (matches the uint16 pair transpose in the AV stage)
            for ot in range(CT):
                for j2 in range(N // 1024):
                    ps = ps_big.tile([128, 1024], f32, tag="psbig")
                    for jc in range(2):
                        for c2 in range(2):
                            nc.tensor.matmul(
                                ps[:, jc * 512 : (jc + 1) * 512],
                                wk_s[:, 2 * c2 : 2 * c2 + 2, ot * 128 : (ot + 1) * 128],
                                x8[
                                    :,
                                    2 * c2 : 2 * c2 + 2,
                                    j2 * 1024 + jc * 512 : j2 * 1024 + (jc + 1) * 512,
                                ],
                                start=(c2 == 0),
                                stop=(c2 == 1),
                                perf_mode=DROW,
                            )
                    dst = k_sb[:, ot, j2 * 1024 : (j2 + 1) * 1024].rearrange(
                        "p (blk t two) -> p blk two t", blk=4, t=128, two=2
                    )
                    if j2 % 2 == 0:
                        nc.scalar.activation(out=dst, in_=ps, func=ACT.Copy)
                    else:
                        nc.vector.tensor_copy(out=dst, in_=ps)

            def qk_part(ib, fine=False):
                """Scores, exp->fp8, denominator, pair transposes."""
                i0 = ib * 128
                p8 = sblk.tile([128, N], fp8, tag="p8")
                denp = loopp.tile([128, 4], f32, tag="denp")
                pT16 = sblk.tile([128, N // 256, 128], u16, tag="pT16")
                p16 = p8[:, :].bitcast(u16)
                for jc in range(N // 1024):
                    ps = ps_big.tile([128, 1024], f32, tag="psbig")
                    for half in range(2):
                        for c2 in range(2):
                            nc.tensor.matmul(
                                ps[:, half * 512 : (half + 1) * 512],
                                q_sb[:, 2 * c2 : 2 * c2 + 2, i0 : i0 + 128],
                                k_sb[
                                    :,
                                    2 * c2 : 2 * c2 + 2,
                                    jc * 1024 + half * 512 : jc * 1024 + (half + 1) * 512,
                                ],
                                start=(c2 == 0),
                                stop=(c2 == 1),
                                perf_mode=DROW,
                            )
                    nc.scalar.activation(
                        out=p8[:, jc * 1024 : (jc + 1) * 1024],
                        in_=ps,
                        func=ACT.Exp,
                        bias=esh,
                        scale=1.0,
                        accum_out=denp[:, jc : jc + 1],
                    )
                    # uint16 pair-view transpose: byte b of u16 (q, t16, i)
                    # holds p[i, col 2*(128*t16+q)+b]; first chunk alone so
                    # the (sequential) AV accumulation starts early. For the
                    # tail blocks, per-chunk transposes shorten the drain.
                    if fine or jc == 0:
                        nc.sync.dma_start_transpose(
                            pT16[:, 4 * jc : 4 * jc + 4, :],
                            p16[:, jc * 512 : (jc + 1) * 512],
                        )
                    elif jc == 3:
                        nc.sync.dma_start_transpose(
                            pT16[:, 4:16, :], p16[:, 512:2048]
                        )
                den = loopp.tile([128, 1], f32, tag="den")
                nc.vector.reduce_sum(out=den, in_=denp, axis=AX.X)
                rden = loopp.tile([128, 1], f32, tag="rden")
                nc.vector.reciprocal(out=rden, in_=den)
                return pT16, rden

            def av_part(ib, pT16, rden):
                """fp8 DoubleRow AV from the pair view, residual, out."""
                pav = ps_av.tile([128, C], f32, tag="pav")
                pT8v = pT16[:, :, :].bitcast(fp8)  # [128, 16, 256]
                nmm = 0
                for u in range(8):
                    for b in range(2):
                        lhs = pT8v[:, 2 * u : 2 * u + 2, :].rearrange(
                            "p t (i two) -> p t two i", two=2
                        )[:, :, b, :]
                        nc.tensor.matmul(
                            pav,
                            lhs,
                            vw_sb[:, 16 * b + 2 * u : 16 * b + 2 * u + 2, :],
                            start=(nmm == 0),
                            stop=(nmm == 15),
                            perf_mode=DROW,
                        )
                        nmm += 1
                outf = loopp.tile([128, C], bf16, tag="outf")
                nc.vector.scalar_tensor_tensor(
                    out=outf,
                    in0=pav,
                    scalar=rden,
                    in1=xrt[:, ib, :],
                    op0=ALU.mult,
                    op1=ALU.add,
                )
                nc.sync.dma_start(out=OUT[ib], in_=outf)

            # v: tile n <- token block 2*(n%16) + n//16; adjacent n pairs
            # share one [128, 1024] PSUM so copies run at 1024 cols
            for n2 in range(N // 256):
                ps = ps_big.tile([128, 1024], f32, tag="psbig")
                for half in range(2):
                    n = 2 * n2 + half
                    g = 2 * (n % 16) + n // 16
                    for c2 in range(2):
                        nc.tensor.matmul(
                            ps[:, half * 512 : (half + 1) * 512],
                            x8[:, 2 * c2 : 2 * c2 + 2, g * 128 : (g + 1) * 128],
                            wov_s[:, 2 * c2 : 2 * c2 + 2, :],
                            start=(c2 == 0),
                            stop=(c2 == 1),
                            perf_mode=DROW,
                        )
                dst = vw_sb[:, 2 * n2 : 2 * n2 + 2, :]
                if n2 % 2 == 0:
                    nc.scalar.activation(out=dst, in_=ps, func=ACT.Copy)
                else:
                    nc.vector.tensor_copy(out=dst, in_=ps)

            # token-major residual (output bias pre-added on host)
            nc.gpsimd.dma_start(out=xrt, in_=XRT[:, :, :])

            for ib in range(NB // 128):
                pT16_i, rden_i = qk_part(ib, fine=(ib >= 6))
                av_part(ib, pT16_i, rden_i)

    if split_waits:
        _split_sync_waits(nc)
    return nc


def _prep_in_maps(x, gn_gamma, gn_beta, wq, bq, wk, bk, wv, bv, wo, bo):
    import ml_dtypes

    f = np.float32
    bf = ml_dtypes.bfloat16
    xr = np.asarray(x, f).reshape(2, C, N)
    wq_t = np.ascontiguousarray(np.asarray(wq, f).T.astype(bf))
    wk_t = np.ascontiguousarray(np.asarray(wk, f).T.astype(bf))
    wov_t = np.ascontiguousarray((np.asarray(wo, f) @ np.asarray(wv, f)).T.astype(bf))
    bias_o0 = np.asarray(bo, f) + np.asarray(wo, f) @ np.asarray(bv, f)

    f8 = ml_dtypes.float8_e4m3  # matches mybir.dt.float8e4's layout

    def vec(v, dt=f):
        return np.ascontiguousarray(
            np.asarray(v, f).reshape(CT, 128).transpose(1, 0)[:, :, None].astype(dt)
        )

    gam = vec(gn_gamma)
    bet = vec(gn_beta)
    bq_s = vec(np.asarray(bq, f) * QKSCALE)

    cidx = np.arange(C)
    grp = cidx // 16  # (512,)
    gsum = np.zeros((128, CT, G), f)
    gbcast = np.zeros((G, CT, 128), f)
    for ct in range(CT):
        for cl in range(128):
            g = grp[ct * 128 + cl]
            gsum[cl, ct, g] = 1.0 / 16.0  # averages the group's channels
            gbcast[g, ct, cl] = 1.0

    in_maps = []
    for core in range(NCORES):
        b, r = divmod(core, 4)
        xroll = np.ascontiguousarray(np.roll(xr[b], -r * NB, axis=1).reshape(CT, 128, N))
        xres_t = np.ascontiguousarray(
            (xroll.reshape(C, N)[:, :NB].T + bias_o0[None, :])
            .reshape(NB // 128, 128, C)
            .transpose(1, 0, 2)
        )
        in_maps.append(
            {
                "x_f8": xroll.astype(f8),
                "xres_t": xres_t,
                "wq_t": wq_t,
                "wk_t": wk_t,
                "wov_t": wov_t,
                "gamma": gam,
                "beta": bet,
                "bq_s": bq_s,
                "gsum8": gsum.astype(f8),
                "gsumf": gsum,
                "gbcast": gbcast,
            }
        )
    return in_maps


def _assemble(results):
    out = np.empty((2, C, N), np.float32)
    for core in range(NCORES):
        b, r = divmod(core, 4)
        out[b][:, r * NB : (r + 1) * NB] = (
            np.asarray(results[core]["out"]).astype(np.float32).reshape(NB, C).T
        )
    return out.reshape(2, C, 64, 64)


def _run(in_maps, trace=False, trace_kwargs=None):
    from concourse.bass_utils import run_bass_kernel_spmd

    if "nc" not in _cache:
        _cache["nc"] = build()
    kw = {}
    if trace:
        kw = {"trace": True, "trace_kwargs": trace_kwargs or {}}
    return run_bass_kernel_spmd(
        _cache["nc"], in_maps, core_ids=list(range(NCORES)), **kw
    )


def kernel(x, gn_gamma, gn_beta, wq, bq, wk, bk, wv, bv, wo, bo):
    in_maps = _prep_in_maps(x, gn_gamma, gn_beta, wq, bq, wk, bk, wv, bv, wo, bo)
    res = _run(in_maps, trace=False)
    return _assemble(res.results)


# revision 35
# speedup vs baseline: 1.0532x; 1.0308x over previous
"""AttnBlock (GroupNorm -> qkv 1x1 -> softmax attention -> proj -> residual)
for x (2, 512, 64, 64) on 8 Trainium2 NeuronCores.

Sharding: core i handles batch i//4 and query-token block i%4 (1024 of 4096
spatial tokens). k/v are computed per-core over all 4096 tokens (no
collectives). Inputs are token-rolled per core so every core runs the same
SPMD graph with its own query block at token offset 0 (attention is
permutation-invariant over key/value tokens).

GroupNorm is folded into the projections: hn_c = a_c * x_c + d_c with
a_c = gamma_c * rsqrt(var_g + eps), d_c = beta_c - a_c * mu_g, so
q = (Wq diag(a)) x + (Wq d + bq), etc. The attention scale 1/sqrt(C) is
folded into Wq/Wk; k's bias cancels in softmax; v's bias folds into the
output bias, pre-added into the residual tile on the host. Only the fp8
copy of x is ever loaded (2MB): group sums run as indicator matmuls on the
otherwise-idle PE (which also keeps its clock ramped through the head),
and sums of squares split between ScalarE (2048-col Square+accum) and DVE
(mul+reduce), all chasing the DMA. Small inputs ride the Pool engine's DMA
queue so SP only carries x8.

The whole attention pipeline runs at the fp8 matmul rate with zero dtype
conversions: exp writes fp8 directly (shifted by -2 so p stays under
fp8e4's max finite 240; the shift cancels against the accumulated
denominator), the (i,j)->(j,i) transpose runs on the 2-byte DMA XBAR over
a uint16 *pair view* of the fp8 p, and the AV matmul consumes the pair
view via a byte-offset stride-2 DoubleRow lhsT (pair-plane stride 1 is
rejected by the ISA, stride-2 inner columns are fine). The byte pairing
lands token 128b+q (per 256-token block) on (partition q, byte b), so
k_sb is written column-reordered (strided copy out of PSUM) and each vw
tile n holds token block 2*(n%16) + n//16, making every lhsT/rhs pair
line up with no extra data movement. The first query block's QK/exp/
transpose are emitted before the v projection so its AV starts the moment
v lands.
"""

import numpy as np

C = 512          # channels
N = 4096         # spatial tokens (64*64)
NB = 1024        # query tokens per core
G = 32           # groups
CT = 4           # channel tiles of 128
EPS = 1e-6
SCALE = float(C) ** -0.5
QKSCALE = float(C) ** -0.25  # split between q and k so fp8 sees a good range
ESHIFT = -2.0    # exp(s + ESHIFT): keeps p < fp8e4 max finite (240)
NCORES = 8

_cache = {}


def _split_sync_waits(nc, maxw=1):
    """This walrus build encodes at most ~1 sync wait per instruction
    descriptor. Move excess sem waits onto same-engine nops inserted just
    before the instruction (in-order sequencers make this equivalent)."""
    from concourse import mybir

    n = 0
    for fn in nc.m.functions:
        for b in fn.blocks:
            out = []
            for ins in b.instructions:
                si = getattr(ins, "sync_info", None)
                if si is not None and si.on_wait and len(si.on_wait) > maxw:
                    waits = list(si.on_wait)
                    extra, keep = waits[:-maxw], waits[-maxw:]
                    for j in range(0, len(extra), maxw):
                        nop = mybir.InstNoOp(name=f"I-wsp{n}", ins=[], outs=[])
                        n += 1
                        nop.engine = ins.engine
                        nop.sync_info = mybir.SyncInfo(
                            on_wait=extra[j : j + maxw], on_update=[]
                        )
                        out.append(nop)
                    ins.sync_info = mybir.SyncInfo(
                        on_wait=keep, on_update=list(si.on_update)
                    )
                out.append(ins)
            b.instructions = out


def build(split_waits=True):
    import concourse.bass as bass
    import concourse.tile as tile
    from concourse import mybir

    f32 = mybir.dt.float32
    bf16 = mybir.dt.bfloat16
    fp8 = mybir.dt.float8e4
    u16 = mybir.dt.uint16
    AX = mybir.AxisListType
    ALU = mybir.AluOpType
    ACT = mybir.ActivationFunctionType
    DROW = mybir.MatmulPerfMode.DoubleRow

    nc = bass.Bass()
    X8 = nc.declare_dram_parameter("x_f8", [CT, 128, N], fp8, isOutput=False)
    WQ = nc.declare_dram_parameter("wq_t", [C, C], bf16, isOutput=False)
    WK = nc.declare_dram_parameter("wk_t", [C, C], bf16, isOutput=False)
    WOV = nc.declare_dram_parameter("wov_t", [C, C], bf16, isOutput=False)
    GAM = nc.declare_dram_parameter("gamma", [128, CT, 1], f32, isOutput=False)
    BET = nc.declare_dram_parameter("beta", [128, CT, 1], f32, isOutput=False)
    BQS = nc.declare_dram_parameter("bq_s", [128, CT, 1], f32, isOutput=False)
    GS8 = nc.declare_dram_parameter("gsum8", [128, CT, G], fp8, isOutput=False)
    GSF = nc.declare_dram_parameter("gsumf", [128, CT, G], f32, isOutput=False)
    GB = nc.declare_dram_parameter("gbcast", [G, CT, 128], f32, isOutput=False)
    XRT = nc.declare_dram_parameter("xres_t", [128, NB // 128, C], f32, isOutput=False)
    OUT = nc.declare_dram_parameter("out", [NB // 128, 128, C], bf16, isOutput=True)

    w_re = {
        "q": WQ.rearrange("(a p) o -> p a o", p=128),
        "k": WK.rearrange("(a p) o -> p a o", p=128),
        "ov": WOV.rearrange("(a p) o -> p a o", p=128),
    }

    with tile.TileContext(nc) as tc, (
        tc.tile_pool(name="singles", bufs=1)
    ) as singles, (
        tc.tile_pool(name="persist", bufs=1)
    ) as persist, (
        tc.tile_pool(name="ps_big", bufs=3, space="PSUM")
    ) as ps_big, (
        tc.tile_pool(name="xbp", bufs=1)
    ) as xbp, (
        tc.tile_pool(name="wfp", bufs=2)
    ) as wfp, (
        tc.tile_pool(name="statp", bufs=2)
    ) as statp, (
        tc.tile_pool(name="junkp", bufs=2)
    ) as junkp, (
        tc.tile_pool(name="junkv", bufs=2)
    ) as junkv:
        # persistent attention tensors
        xrt = persist.tile([128, NB // 128, C], f32)
        wq_s = persist.tile([128, CT, C], fp8)
        wk_s = persist.tile([128, CT, C], fp8)
        wov_s = persist.tile([128, CT, C], fp8)
        q_sb = persist.tile([128, CT, NB], fp8)
        k_sb = persist.tile([128, CT, N], fp8)
        vw_sb = persist.tile([128, N // 128, C], fp8)
        esh = singles.tile([128, 1], f32)

        with (
            tc.tile_pool(name="ps_st", bufs=1, space="PSUM") as ps_st,
            tc.tile_pool(name="ps_sg", bufs=1, space="PSUM") as ps_sg,
        ):
            # ---- fp8 x: the only copy of x ----
            x8 = xbp.tile([128, CT, N], fp8)
            for ct in range(CT):
                for h in range(2):
                    nc.sync.dma_start(
                        out=x8[:, ct, h * 2048 : (h + 1) * 2048],
                        in_=X8[ct, :, h * 2048 : (h + 1) * 2048],
                    )

            # ---- small inputs on the Pool DMA queue (SP carries only
            # x8); the stats matmul indicator goes first ----
            gs8_t = singles.tile([128, CT, G], fp8)
            nc.gpsimd.dma_start(out=gs8_t, in_=GS8[:, :, :])
            gsf_t = singles.tile([128, CT, G], f32)
            nc.gpsimd.dma_start(out=gsf_t, in_=GSF[:, :, :])
            gb_t = singles.tile([G, CT, 128], f32)
            nc.gpsimd.dma_start(out=gb_t, in_=GB[:, :, :])
            gam_t = singles.tile([128, CT, 1], f32)
            nc.gpsimd.dma_start(out=gam_t, in_=GAM[:, :, :])
            bet_t = singles.tile([128, CT, 1], f32)
            nc.gpsimd.dma_start(out=bet_t, in_=BET[:, :, :])
            bqs_t = singles.tile([128, CT, 1], f32)
            nc.gpsimd.dma_start(out=bqs_t, in_=BQS[:, :, :])
            nc.vector.memset(esh, ESHIFT)

            # group sums on the PE: psg[g, j] accumulates
            # (1/16) * sum_{c in g} x8[c, j (mod 512 partials)];
            # these 32 matmuls also keep the PE clock ramped.
            # sums of squares: ScalarE (2048-col Square+accum, late
            # chunks) / DVE (mul+reduce, early chunks) split.
            psg = ps_sg.tile([128, 512], f32, tag="psg")
            sums2 = statp.tile([128, CT, 4], f32, tag="sums2")
            nc.vector.memset(sums2, 0.0)  # 2048-col passes skip odd slots
            nmm = 0
            for ct in range(CT):
                for h in range(4):
                    chunk = x8[:, ct, h * 1024 : (h + 1) * 1024]
                    for c8 in range(2):
                        nc.tensor.matmul(
                            psg[:G, :],
                            gs8_t[:, ct, :],
                            chunk[:, c8 * 512 : (c8 + 1) * 512],
                            start=(nmm == 0),
                            stop=(nmm == 31),
                        )
                        nmm += 1
                    idx = ct * 4 + h
                    if idx < 6:
                        # DVE squares (its own junk pool: sharing one with
                        # ScalarE's serializes buffer reuse cross-engine)
                        junk8 = junkv.tile([128, 1024], fp8, tag="junk8")
                        nc.vector.tensor_mul(out=junk8, in0=chunk, in1=chunk)
                        nc.vector.reduce_sum(
                            out=sums2[:, ct, h : h + 1], in_=junk8, axis=AX.X
                        )
                    elif idx % 2 == 1:
                        junk = junkp.tile([128, 2048], bf16, tag="junk")
                        nc.scalar.activation(
                            out=junk,
                            in_=x8[:, ct, (h - 1) * 1024 : (h + 1) * 1024],
                            func=ACT.Square,
                            accum_out=sums2[:, ct, h : h + 1],
                        )

            # squares partials -> groups (gsf carries 1/16)
            ps2 = ps_st.tile([128, 4], f32, tag="ps_small")
            for ct in range(CT):
                nc.tensor.matmul(
                    ps2[:G, :],
                    gsf_t[:, ct, :],
                    sums2[:, ct, :],
                    start=(ct == 0),
                    stop=(ct == CT - 1),
                )
            # gst = [mu_g, E2_g] (means over the group's 16*4096 values)
            gst = statp.tile([G, 2], f32, tag="gst")
            nc.vector.reduce_sum(out=gst[:, 0:1], in_=psg[:G, :], axis=AX.X)
            nc.vector.reduce_sum(out=gst[:, 1:2], in_=ps2[:G, :], axis=AX.X)
            nc.scalar.mul(out=gst, in_=gst, mul=1.0 / N)
            gvar = statp.tile([G, 1], f32, tag="gvar")
            nc.vector.tensor_mul(out=gvar, in0=gst[:, 0:1], in1=gst[:, 0:1])
            nc.vector.tensor_sub(out=gvar, in0=gst[:, 1:2], in1=gvar)
            eps_t = statp.tile([G, 1], f32, tag="eps")
            nc.vector.memset(eps_t, EPS)
            gsq = statp.tile([G, 1], f32, tag="gsq")
            nc.scalar.activation(
                out=gsq, in_=gvar, func=ACT.Sqrt, bias=eps_t, scale=1.0
            )
            gstat2 = statp.tile([G, 2], f32, tag="gstat2")
            nc.vector.reciprocal(out=gstat2[:, 1:2], in_=gsq)
            nc.vector.tensor_copy(out=gstat2[:, 0:1], in_=gst[:, 0:1])

            # broadcast groups -> channels: mu_inv (128, CT, 2); the 4
            # matmuls land in disjoint columns of one PSUM tile (no WAR,
            # so they pipeline), then one strided copy extracts all four
            mu_inv = statp.tile([128, CT, 2], f32, tag="mu_inv")
            psb = ps_st.tile([128, CT, 4], f32, tag="ps_small")
            for ct in range(CT):
                nc.tensor.matmul(
                    psb[:, ct, 0:2], gb_t[:, ct, :], gstat2,
                    start=True, stop=True,
                )
            nc.vector.tensor_copy(out=mu_inv, in_=psb[:, :, 0:2])

            # a = gamma * inv ; d = beta - a * mu ; aq = a * SCALE
            a_t = statp.tile([128, CT, 1], f32, tag="a_t")
            nc.vector.tensor_mul(out=a_t, in0=gam_t, in1=mu_inv[:, :, 1:2])
            d_t = statp.tile([128, CT, 1], f32, tag="d_t")
            nc.vector.tensor_mul(out=d_t, in0=a_t, in1=mu_inv[:, :, 0:1])
            nc.vector.tensor_sub(out=d_t, in0=bet_t, in1=d_t)
            aq_t = statp.tile([128, CT, 1], f32, tag="aq_t")
            nc.scalar.mul(out=aq_t, in_=a_t, mul=QKSCALE)
            d_bf = statp.tile([128, CT, 1], bf16, tag="d_bf")
            nc.vector.tensor_copy(out=d_bf, in_=d_t)

            # stream q/k/v weights: fold + q bias projection
            bias_q = statp.tile([128, CT, 1], f32, tag="bias_q")
            for wname, wdst, scal, bvec, bdst, bscale in (
                ("q", wq_s, aq_t, d_bf, bias_q, QKSCALE),
                ("k", wk_s, aq_t, None, None, None),
                ("ov", wov_s, a_t, None, None, None),
            ):
                wf = wfp.tile([128, CT, C], bf16, tag="wf")
                nc.gpsimd.dma_start(out=wf, in_=w_re[wname])
                for ct in range(CT):
                    # fold split across DVE and ScalarE (per-partition
                    # scale rides the activation)
                    if ct < 2:
                        nc.vector.tensor_scalar_mul(
                            out=wdst[:, ct, :],
                            in0=wf[:, ct, :],
                            scalar1=scal[:, ct, :],
                        )
                    else:
                        nc.scalar.activation(
                            out=wdst[:, ct, :],
                            in_=wf[:, ct, :],
                            func=ACT.Copy,
                            scale=scal[:, ct, :],
                        )
                if bvec is not None:
                    for ot in range(CT):
                        pb = ps_st.tile([128, 4], f32, tag="ps_small")
                        for ct in range(CT):
                            nc.tensor.matmul(
                                pb[:, 0:1],
                                wf[:, ct, ot * 128 : (ot + 1) * 128],
                                bvec[:, ct, :],
                                start=(ct == 0),
                                stop=(ct == CT - 1),
                            )
                        nc.scalar.activation(
                            out=bdst[:, ot, :],
                            in_=pb[:, 0:1],
                            func=ACT.Identity,
                            bias=bqs_t[:, ot, :],
                            scale=bscale,
                        )

        # ---- projections + attention (small-PSUM pools closed above
        # free the banks ps_av needs) ----
        with (
            tc.tile_pool(name="loopp", bufs=3) as loopp,
            tc.tile_pool(name="sblk", bufs=3) as sblk,
            tc.tile_pool(name="ps_av", bufs=2, space="PSUM") as ps_av,
        ):
            # q: per ot one [128, 1024] PSUM, bias applied on read-out
            for ot in range(CT):
                ps = ps_big.tile([128, 1024], f32, tag="psbig")
                for jc in range(2):
                    for c2 in range(2):
                        nc.tensor.matmul(
                            ps[:, jc * 512 : (jc + 1) * 512],
                            wq_s[:, 2 * c2 : 2 * c2 + 2, ot * 128 : (ot + 1) * 128],
                            x8[:, 2 * c2 : 2 * c2 + 2, jc * 512 : (jc + 1) * 512],
                            start=(c2 == 0),
                            stop=(c2 == 1),
                            perf_mode=DROW,
                        )
                nc.scalar.activation(
                    out=q_sb[:, ot, :],
                    in_=ps,
                    func=ACT.Identity,
                    bias=bias_q[:, ot, :],
                    scale=1.0,
                )

            # k: [128, 1024] PSUM chunks; copy out column-reordered so
            # k_sb column 256B + 2t + b holds token 256B + 128b + t
            # (matches the uint16 pair transpose in the AV stage)
            for ot in range(CT):
                for j2 in range(N // 1024):
                    ps = ps_big.tile([128, 1024], f32, tag="psbig")
                    for jc in range(2):
                        for c2 in range(2):
                            nc.tensor.matmul(
                                ps[:, jc * 512 : (jc + 1) * 512],
                                wk_s[:, 2 * c2 : 2 * c2 + 2, ot * 128 : (ot + 1) * 128],
                                x8[
                                    :,
                                    2 * c2 : 2 * c2 + 2,
                                    j2 * 1024 + jc * 512 : j2 * 1024 + (jc + 1) * 512,
                                ],
                                start=(c2 == 0),
                                stop=(c2 == 1),
                                perf_mode=DROW,
                            )
                    dst = k_sb[:, ot, j2 * 1024 : (j2 + 1) * 1024].rearrange(
                        "p (blk t two) -> p blk two t", blk=4, t=128, two=2
                    )
                    if j2 % 2 == 0:
                        nc.scalar.activation(out=dst, in_=ps, func=ACT.Copy)
                    else:
                        nc.vector.tensor_copy(out=dst, in_=ps)

            def qk_part(ib, fine=False):
                """Scores, exp->fp8, denominator, pair transposes."""
                i0 = ib * 128
                p8 = sblk.tile([128, N], fp8, tag="p8")
                denp = loopp.tile([128, 4], f32, tag="denp")
                pT16 = sblk.tile([128, N // 256, 128], u16, tag="pT16")
                p16 = p8[:, :].bitcast(u16)
                for jc in range(N // 1024):
                    ps = ps_big.tile([128, 1024], f32, tag="psbig")
                    for half in range(2):
                        for c2 in range(2):
                            nc.tensor.matmul(
                                ps[:, half * 512 : (half + 1) * 512],
                                q_sb[:, 2 * c2 : 2 * c2 + 2, i0 : i0 + 128],
                                k_sb[
                                    :,
                                    2 * c2 : 2 * c2 + 2,
                                    jc * 1024 + half * 512 : jc * 1024 + (half + 1) * 512,
                                ],
                                start=(c2 == 0),
                                stop=(c2 == 1),
                                perf_mode=DROW,
                            )
                    nc.scalar.activation(
                        out=p8[:, jc * 1024 : (jc + 1) * 1024],
                        in_=ps,
                        func=ACT.Exp,
                        bias=esh,
                        scale=1.0,
                        accum_out=denp[:, jc : jc + 1],
                    )
                    # uint16 pair-view transpose: byte b of u16 (q, t16, i)
                    # holds p[i, col 2*(128*t16+q)+b]; first chunk alone so
                    # the (sequential) AV accumulation starts early. For the
                    # tail blocks, per-chunk transposes shorten the drain.
                    if fine or jc == 0:
                        nc.sync.dma_start_transpose(
                            pT16[:, 4 * jc : 4 * jc + 4, :],
                            p16[:, jc * 512 : (jc + 1) * 512],
                        )
                    elif jc == 3:
                        nc.sync.dma_start_transpose(
                            pT16[:, 4:16, :], p16[:, 512:2048]
                        )
                den = loopp.tile([128, 1], f32, tag="den")
                nc.vector.reduce_sum(out=den, in_=denp, axis=AX.X)
                rden = loopp.tile([128, 1], f32, tag="rden")
                nc.vector.reciprocal(out=rden, in_=den)
                return pT16, rden

            def av_part(ib, pT16, rden):
                """fp8 DoubleRow AV from the pair view, residual, out."""
                pav = ps_av.tile([128, C], f32, tag="pav")
                pT8v = pT16[:, :, :].bitcast(fp8)  # [128, 16, 256]
                nmm = 0
                for u in range(8):
                    for b in range(2):
                        lhs = pT8v[:, 2 * u : 2 * u + 2, :].rearrange(
                            "p t (i two) -> p t two i", two=2
                        )[:, :, b, :]
                        nc.tensor.matmul(
                            pav,
                            lhs,
                            vw_sb[:, 16 * b + 2 * u : 16 * b + 2 * u + 2, :],
                            start=(nmm == 0),
                            stop=(nmm == 15),
                            perf_mode=DROW,
                        )
                        nmm += 1
                outf = loopp.tile([128, C], bf16, tag="outf")
                nc.vector.scalar_tensor_tensor(
                    out=outf,
                    in0=pav,
                    scalar=rden,
                    in1=xrt[:, ib, :],
                    op0=ALU.mult,
                    op1=ALU.add,
                )
                nc.sync.dma_start(out=OUT[ib], in_=outf)

            # v: tile n <- token block 2*(n%16) + n//16; adjacent n pairs
            # share one [128, 1024] PSUM so copies run at 1024 cols
            for n2 in range(N // 256):
                ps = ps_big.tile([128, 1024], f32, tag="psbig")
                for half in range(2):
                    n = 2 * n2 + half
                    g = 2 * (n % 16) + n // 16
                    for c2 in range(2):
                        nc.tensor.matmul(
                            ps[:, half * 512 : (half + 1) * 512],
                            x8[:, 2 * c2 : 2 * c2 + 2, g * 128 : (g + 1) * 128],
                            wov_s[:, 2 * c2 : 2 * c2 + 2, :],
                            start=(c2 == 0),
                            stop=(c2 == 1),
                            perf_mode=DROW,
                        )
                dst = vw_sb[:, 2 * n2 : 2 * n2 + 2, :]
                if n2 % 2 == 0:
                    nc.scalar.activation(out=dst, in_=ps, func=ACT.Copy)
                else:
                    nc.vector.tensor_copy(out=dst, in_=ps)

            # token-major residual (output bias pre-added on host)
            nc.gpsimd.dma_start(out=xrt, in_=XRT[:, :, :])

            for ib in range(NB // 128):
                pT16_i, rden_i = qk_part(ib)
                av_part(ib, pT16_i, rden_i)

    if split_waits:
        _split_sync_waits(nc)
    return nc


def _prep_in_maps(x, gn_gamma, gn_beta, wq, bq, wk, bk, wv, bv, wo, bo):
    import ml_dtypes

    f = np.float32
    bf = ml_dtypes.bfloat16
    xr = np.asarray(x, f).reshape(2, C, N)
    wq_t = np.ascontiguousarray(np.asarray(wq, f).T.astype(bf))
    wk_t = np.ascontiguousarray(np.asarray(wk, f).T.astype(bf))
    wov_t = np.ascontiguousarray((np.asarray(wo, f) @ np.asarray(wv, f)).T.astype(bf))
    bias_o0 = np.asarray(bo, f) + np.asarray(wo, f) @ np.asarray(bv, f)

    f8 = ml_dtypes.float8_e4m3  # matches mybir.dt.float8e4's layout

    def vec(v, dt=f):
        return np.ascontiguousarray(
            np.asarray(v, f).reshape(CT, 128).transpose(1, 0)[:, :, None].astype(dt)
        )

    gam = vec(gn_gamma)
    bet = vec(gn_beta)
    bq_s = vec(np.asarray(bq, f) * QKSCALE)

    cidx = np.arange(C)
    grp = cidx // 16  # (512,)
    gsum = np.zeros((128, CT, G), f)
    gbcast = np.zeros((G, CT, 128), f)
    for ct in range(CT):
        for cl in range(128):
            g = grp[ct * 128 + cl]
            gsum[cl, ct, g] = 1.0 / 16.0  # averages the group's channels
            gbcast[g, ct, cl] = 1.0

    in_maps = []
    for core in range(NCORES):
        b, r = divmod(core, 4)
        xroll = np.ascontiguousarray(np.roll(xr[b], -r * NB, axis=1).reshape(CT, 128, N))
        xres_t = np.ascontiguousarray(
            (xroll.reshape(C, N)[:, :NB].T + bias_o0[None, :])
            .reshape(NB // 128, 128, C)
            .transpose(1, 0, 2)
        )
        in_maps.append(
            {
                "x_f8": xroll.astype(f8),
                "xres_t": xres_t,
                "wq_t": wq_t,
                "wk_t": wk_t,
                "wov_t": wov_t,
                "gamma": gam,
                "beta": bet,
                "bq_s": bq_s,
                "gsum8": gsum.astype(f8),
                "gsumf": gsum,
                "gbcast": gbcast,
            }
        )
    return in_maps


def _assemble(results):
    out = np.empty((2, C, N), np.float32)
    for core in range(NCORES):
        b, r = divmod(core, 4)
        out[b][:, r * NB : (r + 1) * NB] = (
            np.asarray(results[core]["out"]).astype(np.float32).reshape(NB, C).T
        )
    return out.reshape(2, C, 64, 64)


def _run(in_maps, trace=False, trace_kwargs=None):
    from concourse.bass_utils import run_bass_kernel_spmd

    if "nc" not in _cache:
        _cache["nc"] = build()
    kw = {}
    if trace:
        kw = {"trace": True, "trace_kwargs": trace_kwargs or {}}
    return run_bass_kernel_spmd(
        _cache["nc"], in_maps, core_ids=list(range(NCORES)), **kw
    )


def kernel(x, gn_gamma, gn_beta, wq, bq, wk, bk, wv, bv, wo, bo):
    in_maps = _prep_in_maps(x, gn_gamma, gn_beta, wq, bq, wk, bk, wv, bv, wo, bo)
    res = _run(in_maps, trace=False)
    return _assemble(res.results)
